# revision 32
# baseline (speedup 1.0000x reference)
"""GATConv Trainium kernel (single-core SPMD program) + host prep.  V2.

Per-core program (identical NEFF on all 8 cores, different input data):
  Node tables are ROTATED per core: table row r = global node
  (dev_base + r) % N, so every core's own nodes are rows 0..DEV_N-1 and the
  program stays core-independent. The host rotates xT and all indices.

  Phase 1 (all V rows): one packed bf16 table
  h_ext[r, 0:136] = [h = x@W.T (128) | a_src (4) | a_dst (4)], 256-wide bf16
  rows (512B, dma_gather elem multiple of 256B; cols 136:256 unwritten junk,
  never read).

  Phase 2, per dst-block (128 own nodes), edges pre-routed/sorted by host:
  - h-gather: full 512B rows of h_ext by src (int16 idx; lo section src <
    32768 from h_ext[0:], hi section src-32768 from h_ext[32768:]) -> stage
    [e, t, 256]: h at 0:128, a_src at 128:132.
  - a-gather: 256B half-rows h_ext[:, 128:256] by dst row (= local dst,
    rows 0..DEV_N-1, single section) -> astage [e, t, 128]: a_dst at 4:8.
  - ea = exp(leaky(a_src[src] + a_dst[dst])) (bf16), Gs = h[src]*ea.
  - rhs[e, t, 0:260] = [Gs(128) | ea(4) | h(128)]; one-hot
    sel[e, m] = (dst_loc[e] == m) in bf16; PSUM acc accumulates
    sel.T @ rhs over the block's T tiles => [P | s | Q].
  - out = P/s + Q.

Edge layout: per block, lo-section edges then hi-section edges, sorted by
src within each section (DRAM row locality for the gather), each padded to
global fixed tile counts (T_LO / T_HI) with idx-0 edges carrying
dst_loc = -1 (zero one-hot row => no contribution). Edge i of a section is
at (lane = i%128, tile = i//128); dma_gather's index j lives at
idx16[j%16, j//16], replicated 8x down the 128 partitions.
"""

import numpy as np

import concourse.bass as bass
import concourse.bacc as bacc
import concourse.mybir as mybir
import concourse.tile as tile
from concourse import library_config

DT = mybir.dt
ALU = mybir.AluOpType
ACTF = mybir.ActivationFunctionType

F = 128    # feature dim (in == out)
NH = 4     # heads
HD = 32    # head dim = 32
HEC = 136  # used h_ext cols: h(128) | a_src(4) | a_dst(4)
GE = 256   # h_ext row elems (bf16 -> 512B, mult of 256B)
AE = 128   # a-gather elem width (bf16 -> 256B)
RC = 260   # rhs per-tile block: Gs(128) | ea(4) | h(128)


def build_gat_nc(V, DEV_N, T_LO, T_HI, HALF=32768, leaky=0.2):
    """Build the single-core Bass program."""
    T = T_LO + T_HI
    NBLK = (DEV_N + 127) // 128

    nc = bacc.Bacc(num_swdge_queues=4)
    xT16 = nc.declare_dram_parameter("xT16", [F, V], DT.bfloat16,
                                     isOutput=False)
    Wnat = nc.declare_dram_parameter("Wnat", [F, F], DT.float32,
                                     isOutput=False)
    Wt = nc.declare_dram_parameter("Wt", [F, F], DT.float32, isOutput=False)
    Aatt = nc.declare_dram_parameter("Aatt", [F, 2 * NH], DT.float32,
                                     isOutput=False)
    gidx = nc.declare_dram_parameter("gidx", [128, NBLK * T * 8], DT.int16,
                                     isOutput=False)
    dstLb = nc.declare_dram_parameter("dstLb", [128, NBLK * T * 128],
                                      DT.int16, isOutput=False)
    dstL = nc.declare_dram_parameter("dstL", [128, NBLK * T], DT.int16,
                                     isOutput=False)
    out = nc.declare_dram_parameter("out", [DEV_N, F], DT.float32,
                                    isOutput=True)

    h_ext = nc.dram_tensor("h_ext", [V, GE], DT.bfloat16)

    with tile.TileContext(nc) as tc:
        with (
            tc.tile_pool(name="const", bufs=1) as const,
            tc.tile_pool(name="p1", bufs=3) as p1,
            tc.tile_pool(name="p1ps", bufs=2, space="PSUM") as p1ps,
            tc.tile_pool(name="p2", bufs=2) as p2,
            tc.tile_pool(name="pb", bufs=3) as pb,
            tc.tile_pool(name="prhs", bufs=2) as prhs,
            tc.tile_pool(name="p2ps", bufs=2, space="PSUM") as p2ps,
        ):
            nc.gpsimd.load_library(library_config.mlp)

            # ---- constants ----
            wnat_t = const.tile([128, F], DT.float32)
            aatt_t = const.tile([128, 2 * NH], DT.float32)
            wt_t = const.tile([128, F], DT.float32)
            wext16 = const.tile([128, HEC], DT.bfloat16)
            iota32 = const.tile([128, 128], DT.int32)
            iota16 = const.tile([128, 128], DT.int16)
            iotac32 = const.tile([128, 1], DT.int32)
            iotac16 = const.tile([128, 1], DT.int16)
            # per-partition index replicated along free dim, for selT build
            iota_cw = const.tile([128, T * 128], DT.int16)
            leak_c = const.tile([128, 1], DT.float32)
            nc.gpsimd.memset(leak_c[:], leaky)
            nc.sync.dma_start(out=wnat_t[:], in_=Wnat[:, :])
            nc.sync.dma_start(out=aatt_t[:], in_=Aatt[:, :])
            nc.sync.dma_start(out=wt_t[:], in_=Wt[:, :])
            nc.gpsimd.iota(iota32[:], pattern=[[1, 128]], base=0,
                           channel_multiplier=0)
            nc.vector.tensor_copy(out=iota16[:], in_=iota32[:])
            nc.gpsimd.iota(iotac32[:], pattern=[[0, 1]], base=0,
                           channel_multiplier=1)
            nc.vector.tensor_copy(out=iotac16[:], in_=iotac32[:])
            nc.vector.tensor_copy(
                out=iota_cw[:],
                in_=bass.AP(iotac16[:].tensor, 0, [[1, 128], [0, T * 128]]))
            vps = p1ps.tile([128, 2 * NH], DT.float32, tag="vps")
            nc.tensor.matmul(out=vps[:], lhsT=wnat_t[:], rhs=aatt_t[:],
                             start=True, stop=True)
            nc.vector.tensor_copy(out=wext16[:, 0:F], in_=wt_t[:])
            nc.vector.tensor_copy(out=wext16[:, F:HEC], in_=vps[:])

            # ---- phase 1 (batches of 8 node chunks) ----
            nchunks = (V + 127) // 128
            CBATCH = 8
            for cb in range(0, nchunks, CBATCH):
                nb = min(CBATCH, nchunks - cb)
                c0 = cb * 128
                nn = min(V - c0, nb * 128)
                xc = p1.tile([128, CBATCH * 128], DT.bfloat16, tag="xc")
                nc.scalar.dma_start(out=xc[:, :nn], in_=xT16[:, c0:c0 + nn])
                hrow = p1.tile([128, CBATCH * GE], DT.bfloat16, tag="hrow")
                for k in range(nb):
                    m = min(128, V - (c0 + k * 128))
                    hps = p1ps.tile([128, HEC], DT.float32, tag="hps")
                    nc.tensor.matmul(
                        out=hps[:m, :],
                        lhsT=xc[:, k * 128:k * 128 + m],
                        rhs=wext16[:],
                        start=True, stop=True)
                    nc.vector.tensor_copy(
                        out=hrow[:m, k * GE:k * GE + HEC],
                        in_=hps[:m, :])
                # contiguous full-row batched writes (junk cols included;
                # never read)
                last = min(V, c0 + nb * 128)
                kfull = (last - c0) // 128  # full 128-row chunks in batch
                if kfull > 0:
                    nc.sync.dma_start(
                        out=bass.AP(h_ext[:, :].tensor, c0 * GE,
                                    [[GE, 128], [GE * 128, kfull], [1, GE]]),
                        in_=hrow[:].rearrange("p (k c) -> p k c", c=GE)[
                            :, 0:kfull, :])
                for k in range(kfull, nb):
                    m = min(128, V - (c0 + k * 128))
                    nc.sync.dma_start(
                        out=h_ext[c0 + k * 128:c0 + k * 128 + m, :],
                        in_=hrow[:m, k * GE:(k + 1) * GE])

            # ---- phase 2 ----
            qs = [0, 1, 2, 3]
            for b in range(NBLK):
                rows = min(128, DEV_N - b * 128)
                bT8 = b * T * 8
                dl = p2.tile([128, T], DT.int16, tag="dl")
                nc.sync.dma_start(out=dl[:], in_=dstL[:, b * T:(b + 1) * T])
                gi = p2.tile([128, T * 8], DT.int16, tag="gi")
                nc.sync.dma_start(out=gi[:], in_=gidx[:, bT8:bT8 + T * 8])
                dlb = p2.tile([128, T * 128], DT.int16, tag="dlb")
                nc.sync.dma_start(
                    out=dlb[:],
                    in_=dstLb[:, b * T * 128:(b + 1) * T * 128])
                a_blk = p2.tile([128, 2 * NH], DT.bfloat16, tag="a_blk")
                nc.sync.dma_start(out=a_blk[:],
                                  in_=h_ext[b * 128:(b + 1) * 128, F:F + 2 * NH])

                stage = pb.tile([128, T * GE], DT.bfloat16, tag="stage")
                sr = stage[:].rearrange("p (t g) -> p t g", g=GE)
                nc.gpsimd.dma_gather(
                    out_ap=sr[:, 0:T_LO, :],
                    in_ap=h_ext[0:, :],
                    idxs_ap=gi[:, 0:T_LO * 8],
                    num_idxs=T_LO * 128, num_idxs_reg=T_LO * 128,
                    elem_size=GE, single_packet=False,
                    queue_num=qs[(2 * b) % 4])
                nc.gpsimd.dma_gather(
                    out_ap=sr[:, T_LO:T, :],
                    in_ap=h_ext[HALF:, :],
                    idxs_ap=gi[:, T_LO * 8:T * 8],
                    num_idxs=T_HI * 128, num_idxs_reg=T_HI * 128,
                    elem_size=GE, single_packet=False,
                    queue_num=qs[(2 * b + 1) % 4])

                # transposed one-hot selT[m, (t, e)] = (dstL[e, t] == m), bf16
                selT = prhs.tile([128, T * 128], DT.bfloat16, tag="selT")
                selTr = selT[:].rearrange("p (t e) -> p t e", e=128)
                nc.vector.tensor_tensor(
                    out=selTr[:, 0:T, :],
                    in0=dlb[:].rearrange("p (t e) -> p t e", e=128),
                    in1=iota_cw[:].rearrange("p (t e) -> p t e", e=128),
                    op=ALU.is_equal)
                # a_dst[e, h] per edge via one-hot matmul
                par = p2ps.tile([128, T * NH], DT.float32, tag="par")
                parr = par[:].rearrange("p (t e) -> p t e", e=NH)
                for j in range(T):
                    nc.tensor.matmul(
                        out=parr[:, j, :], lhsT=selTr[:, j, :],
                        rhs=a_blk[:, NH:2 * NH], start=True, stop=True)

                # one-hot sel[e, (t, m)] = (dstL[e, t] == m), bf16
                sel = prhs.tile([128, T * 128], DT.bfloat16, tag="sel")
                selr = sel[:].rearrange("p (t m) -> p t m", m=128)
                nc.vector.tensor_tensor(
                    out=selr[:, 0:T, :],
                    in0=dl[:][:, :, None].to_broadcast([128, T, 128]),
                    in1=iota16[:][:, None, :].to_broadcast([128, T, 128]),
                    op=ALU.is_equal)

                rhs = prhs.tile([128, T * RC], DT.bfloat16, tag="rhs")
                rr = rhs[:].rearrange("p (t c) -> p t c", c=RC)

                # ea chain: alpha -> leaky -> exp into rhs[:, :, 128:132]
                scr = p2.tile([128, T * NH], DT.float32, tag="scr")
                scrr = scr[:].rearrange("p (t e) -> p t e", e=NH)
                nc.vector.tensor_tensor(
                    out=scrr[:, 0:T, :], in0=sr[:, 0:T, F:F + NH],
                    in1=parr[:, 0:T, :], op=ALU.add)
                scr2 = p2.tile([128, T * NH], DT.float32, tag="scr2")
                scr2r = scr2[:].rearrange("p (t e) -> p t e", e=NH)
                nc.vector.tensor_tensor(
                    out=scr2r[:, 0:T, :], in0=scrr[:, 0:T, :],
                    in1=bass.AP(leak_c[:].tensor, 0,
                                [[1, 128], [0, T], [0, NH]]),
                    op=ALU.mult)
                nc.vector.tensor_tensor(
                    out=scrr[:, 0:T, :], in0=scrr[:, 0:T, :],
                    in1=scr2r[:, 0:T, :], op=ALU.max)
                nc.scalar.activation(out=rr[:, 0:T, F:F + NH],
                                     in_=scrr[:, 0:T, :], func=ACTF.Exp)

                # h copy on scalar engine
                nc.scalar.copy(out=rr[:, 0:T, F + NH:RC],
                               in_=sr[:, 0:T, 0:F])
                # expand ea along head dim into rhs[:, :, 0:F] (scalar
                # engine), then Gs = h * ea at full 16-bit DVE rate
                nc.scalar.copy(
                    out=rr[:, 0:T, 0:F].rearrange(
                        "p t (h e) -> p t h e", e=HD),
                    in_=rr[:, 0:T, F:F + NH][:, :, :, None].to_broadcast(
                        [128, T, NH, HD]))
                nc.vector.tensor_tensor(
                    out=rr[:, 0:T, 0:F],
                    in0=sr[:, 0:T, 0:F],
                    in1=rr[:, 0:T, 0:F],
                    op=ALU.mult)

                # accumulate
                acc = p2ps.tile([128, RC], DT.float32, tag="acc")
                for j in range(T):
                    nc.tensor.matmul(
                        out=acc[:], lhsT=selr[:, j, :], rhs=rr[:, j, :],
                        start=(j == 0), stop=(j == T - 1))

                # ---- evac: out = P / s + Q ----
                sden = p2.tile([128, NH], DT.float32, tag="sden")
                nc.vector.tensor_scalar_max(out=sden[:], in0=acc[:, F:F + NH],
                                            scalar1=1e-30)
                rs = p2.tile([128, NH], DT.float32, tag="rs")
                nc.vector.reciprocal(out=rs[:], in_=sden[:])
                ot = p2.tile([128, F], DT.float32, tag="ot")
                otr = ot[:].rearrange("p (h e) -> p h e", e=HD)
                nc.vector.tensor_tensor(
                    out=otr,
                    in0=acc[:, 0:F].rearrange("p (h e) -> p h e", e=HD),
                    in1=rs[:][:, :, None].to_broadcast([128, NH, HD]),
                    op=ALU.mult)
                nc.vector.tensor_tensor(
                    out=otr, in0=otr,
                    in1=acc[:, F + NH:RC].rearrange("p (h e) -> p h e", e=HD),
                    op=ALU.add)
                nc.sync.dma_start(out=out[b * 128:b * 128 + rows, :],
                                  in_=ot[:rows, :])

    return nc


def route_edges(edge_index, N, n_cores, half=32768):
    """Host edge routing. Returns (T_LO, T_HI, per_core index dicts)."""
    src = np.concatenate([np.asarray(edge_index[0]),
                          np.arange(N)]).astype(np.int64)
    dst = np.concatenate([np.asarray(edge_index[1]),
                          np.arange(N)]).astype(np.int64)
    dev_n = N // n_cores
    assert dev_n * n_cores == N
    core = dst // dev_n
    nblk = (dev_n + 127) // 128

    per_core_raw = []
    T_LO = T_HI = 0
    for d in range(n_cores):
        m = core == d
        s_rot = (src[m] - d * dev_n) % N
        d_loc = dst[m] - d * dev_n
        blk = d_loc // 128
        lo = s_rot < half
        cnt_lo = np.bincount(blk[lo], minlength=nblk)
        cnt_hi = np.bincount(blk[~lo], minlength=nblk)
        T_LO = max(T_LO, int(-(-cnt_lo.max() // 128)))
        T_HI = max(T_HI, int(-(-cnt_hi.max() // 128)))
        per_core_raw.append((s_rot, d_loc, blk, lo))
    T_HI = max(T_HI, 1)
    T_LO = max(T_LO, 1)
    T = T_LO + T_HI

    per_core = []
    for d in range(n_cores):
        s_rot, d_loc, blk, lo = per_core_raw[d]
        gidx16 = np.zeros((16, nblk * T * 8), dtype=np.int16)
        dstL = np.full((128, nblk * T), -1, dtype=np.int16)
        for b in range(nblk):
            bcol = b * T * 8
            for sec in (0, 1):
                if sec == 0:
                    bm = (blk == b) & lo
                    voff, t0, sec_col = 0, 0, bcol
                else:
                    bm = (blk == b) & ~lo
                    voff, t0, sec_col = half, T_LO, bcol + T_LO * 8
                vals = s_rot[bm]
                dloc_b = d_loc[bm]
                n = len(vals)
                if n == 0:
                    continue
                order = np.argsort(vals, kind="stable")
                vals = vals[order] - voff
                dloc_b = dloc_b[order]
                jj = np.arange(n)
                gidx16[jj % 16, sec_col + jj // 16] = vals.astype(np.int16)
                dstL[jj % 128, b * T + t0 + jj // 128] = (
                    dloc_b - b * 128).astype(np.int16)

        # dstLb[m, (b, t, e)] = dstL[e, b*T + t], replicated down partitions
        dstLb = np.ascontiguousarray(np.broadcast_to(
            dstL.T.reshape(1, -1), (128, nblk * T * 128)).astype(np.int16))
        per_core.append({
            "gidx": np.tile(gidx16, (8, 1)),
            "dstLb": dstLb,
            "dstL": dstL,
        })
    return T_LO, T_HI, per_core


def host_prep(x, edge_index, W, att_src, att_dst, n_cores, half=32768):
    """Returns (T_LO, T_HI, per-core in_maps list)."""
    N = x.shape[0]
    dev_n = N // n_cores
    bf16 = DT.np(DT.bfloat16)
    xTf = np.ascontiguousarray(np.asarray(x).T.astype(np.float32))
    Wnat = np.ascontiguousarray(np.asarray(W).astype(np.float32))
    Wt = np.ascontiguousarray(Wnat.T)
    A = np.zeros((F, 2 * NH), dtype=np.float32)
    for h in range(NH):
        A[h * HD:(h + 1) * HD, h] = np.asarray(att_src)[0, h]
        A[h * HD:(h + 1) * HD, NH + h] = np.asarray(att_dst)[0, h]
    T_LO, T_HI, per_core = route_edges(edge_index, N, n_cores, half)
    in_maps = []
    for d in range(n_cores):
        xr = np.roll(xTf, -d * dev_n, axis=1)
        in_maps.append(dict(per_core[d],
                            xT16=np.ascontiguousarray(xr.astype(bf16)),
                            Wnat=Wnat, Wt=Wt, Aatt=A))
    return T_LO, T_HI, in_maps


# ---------------------------------------------------------------------------
# Self-contained kernel entry point (full problem size hardcoded).
# ---------------------------------------------------------------------------
N_NODES = 50000
N_CORES = 8
HALF_SPLIT = 32768


def _run(inputs, trace=False):
    import time
    from concourse.bass_utils import run_bass_kernel_spmd

    global LAST_RES
    x = np.asarray(inputs["x"], dtype=np.float32)
    edge_index = np.asarray(inputs["edge_index"])
    W = np.asarray(inputs["W"], dtype=np.float32)
    att_src = np.asarray(inputs["att_src"], dtype=np.float32)
    att_dst = np.asarray(inputs["att_dst"], dtype=np.float32)

    N = x.shape[0]
    assert N == N_NODES, N
    dev_n = N // N_CORES

    t0 = time.time()
    T_LO, T_HI, in_maps = host_prep(x, edge_index, W, att_src, att_dst,
                                    N_CORES, half=HALF_SPLIT)
    t1 = time.time()
    nc = build_gat_nc(N, dev_n, T_LO, T_HI, HALF=HALF_SPLIT)
    nc.compile()
    t2 = time.time()
    res = run_bass_kernel_spmd(nc, in_maps, list(range(N_CORES)), trace=trace)
    LAST_RES = res
    t3 = time.time()
    print(f"kernel: host_prep {t1-t0:.1f}s build+compile {t2-t1:.1f}s "
          f"run {t3-t2:.1f}s T_LO={T_LO} T_HI={T_HI}")
    out = np.concatenate([res.results[d]["out"] for d in range(N_CORES)],
                         axis=0).astype(np.float32)
    return out, res.exec_time_ns


def kernel(**inputs) -> np.ndarray:
    return _run(inputs, trace=False)[0]


# revision 35
# speedup vs baseline: 1.1557x; 1.1557x over previous
"""GATConv Trainium kernel (single-core SPMD program) + host prep.  V2.

Per-core program (identical NEFF on all 8 cores, different input data):
  Node tables are ROTATED per core: table row r = global node
  (dev_base + r) % N, so every core's own nodes are rows 0..DEV_N-1 and the
  program stays core-independent. The host rotates xT and all indices.

  Phase 1 (all V rows): one packed bf16 table
  h_ext[r, 0:136] = [h = x@W.T (128) | a_src (4) | a_dst (4)], 256-wide bf16
  rows (512B, dma_gather elem multiple of 256B; cols 136:256 unwritten junk,
  never read).

  Phase 2, per dst-block (128 own nodes), edges pre-routed/sorted by host:
  - h-gather: full 512B rows of h_ext by src (int16 idx; lo section src <
    32768 from h_ext[0:], hi section src-32768 from h_ext[32768:]) -> stage
    [e, t, 256]: h at 0:128, a_src at 128:132.
  - a-gather: 256B half-rows h_ext[:, 128:256] by dst row (= local dst,
    rows 0..DEV_N-1, single section) -> astage [e, t, 128]: a_dst at 4:8.
  - ea = exp(leaky(a_src[src] + a_dst[dst])) (bf16), Gs = h[src]*ea.
  - rhs[e, t, 0:260] = [Gs(128) | ea(4) | h(128)]; one-hot
    sel[e, m] = (dst_loc[e] == m) in bf16; PSUM acc accumulates
    sel.T @ rhs over the block's T tiles => [P | s | Q].
  - out = P/s + Q.

Edge layout: per block, lo-section edges then hi-section edges, sorted by
src within each section (DRAM row locality for the gather), each padded to
global fixed tile counts (T_LO / T_HI) with idx-0 edges carrying
dst_loc = -1 (zero one-hot row => no contribution). Edge i of a section is
at (lane = i%128, tile = i//128); dma_gather's index j lives at
idx16[j%16, j//16], replicated 8x down the 128 partitions.
"""

import numpy as np

import concourse.bass as bass
import concourse.bacc as bacc
import concourse.mybir as mybir
import concourse.tile as tile
from concourse import library_config

DT = mybir.dt
ALU = mybir.AluOpType
ACTF = mybir.ActivationFunctionType

F = 128    # feature dim (in == out)
NH = 4     # heads
HD = 32    # head dim = 32
HEC = 136  # used h_ext cols: h(128) | a_src(4) | a_dst(4)
GE = 256   # h_ext row elems (bf16 -> 512B, mult of 256B)
AE = 128   # a-gather elem width (bf16 -> 256B)
RC = 260   # rhs per-tile block: Gs(128) | ea(4) | h(128)


def build_gat_nc(V, DEV_N, T_LO, T_HI, HALF=32768, leaky=0.2):
    """Build the single-core Bass program."""
    T = T_LO + T_HI
    NBLK = (DEV_N + 127) // 128

    nc = bacc.Bacc(num_swdge_queues=4)
    xT16 = nc.declare_dram_parameter("xT16", [F, V], DT.bfloat16,
                                     isOutput=False)
    Wnat = nc.declare_dram_parameter("Wnat", [F, F], DT.float32,
                                     isOutput=False)
    Wt = nc.declare_dram_parameter("Wt", [F, F], DT.float32, isOutput=False)
    Aatt = nc.declare_dram_parameter("Aatt", [F, 2 * NH], DT.float32,
                                     isOutput=False)
    gidx = nc.declare_dram_parameter("gidx", [128, NBLK * T * 8], DT.int16,
                                     isOutput=False)
    dstLb = nc.declare_dram_parameter("dstLb", [128, NBLK * T * 128],
                                      DT.int16, isOutput=False)
    dstL = nc.declare_dram_parameter("dstL", [128, NBLK * T], DT.int16,
                                     isOutput=False)
    out = nc.declare_dram_parameter("out", [DEV_N, F], DT.float32,
                                    isOutput=True)

    h_ext = nc.dram_tensor("h_ext", [V, GE], DT.bfloat16)

    with tile.TileContext(nc) as tc:
        with (
            tc.tile_pool(name="const", bufs=1) as const,
            tc.tile_pool(name="p1", bufs=3) as p1,
            tc.tile_pool(name="p1ps", bufs=2, space="PSUM") as p1ps,
            tc.tile_pool(name="p2", bufs=4) as p2,
            tc.tile_pool(name="pdlb", bufs=2) as pdlb,
            tc.tile_pool(name="pb", bufs=3) as pb,
            tc.tile_pool(name="prhs", bufs=2) as prhs,
            tc.tile_pool(name="p2ps", bufs=2, space="PSUM") as p2ps,
        ):
            nc.gpsimd.load_library(library_config.mlp)

            # ---- constants ----
            wnat_t = const.tile([128, F], DT.float32)
            aatt_t = const.tile([128, 2 * NH], DT.float32)
            wt_t = const.tile([128, F], DT.float32)
            wext16 = const.tile([128, HEC], DT.bfloat16)
            iota32 = const.tile([128, 128], DT.int32)
            iota16 = const.tile([128, 128], DT.int16)
            iotac32 = const.tile([128, 1], DT.int32)
            iotac16 = const.tile([128, 1], DT.int16)
            # per-partition index replicated along free dim, for selT build
            iota_cw = const.tile([128, T * 128], DT.int16)
            leak_c = const.tile([128, 1], DT.float32)
            nc.gpsimd.memset(leak_c[:], leaky)
            nc.sync.dma_start(out=wnat_t[:], in_=Wnat[:, :])
            nc.sync.dma_start(out=aatt_t[:], in_=Aatt[:, :])
            nc.sync.dma_start(out=wt_t[:], in_=Wt[:, :])
            nc.gpsimd.iota(iota32[:], pattern=[[1, 128]], base=0,
                           channel_multiplier=0)
            nc.vector.tensor_copy(out=iota16[:], in_=iota32[:])
            nc.gpsimd.iota(iotac32[:], pattern=[[0, 1]], base=0,
                           channel_multiplier=1)
            nc.vector.tensor_copy(out=iotac16[:], in_=iotac32[:])
            nc.vector.tensor_copy(
                out=iota_cw[:],
                in_=bass.AP(iotac16[:].tensor, 0, [[1, 128], [0, T * 128]]))
            vps = p1ps.tile([128, 2 * NH], DT.float32, tag="vps")
            nc.tensor.matmul(out=vps[:], lhsT=wnat_t[:], rhs=aatt_t[:],
                             start=True, stop=True)
            nc.vector.tensor_copy(out=wext16[:, 0:F], in_=wt_t[:])
            nc.vector.tensor_copy(out=wext16[:, F:HEC], in_=vps[:])

            # ---- phase 1 (batches of 8 node chunks) ----
            nchunks = (V + 127) // 128
            CBATCH = 8
            for cb in range(0, nchunks, CBATCH):
                nb = min(CBATCH, nchunks - cb)
                c0 = cb * 128
                nn = min(V - c0, nb * 128)
                xc = p1.tile([128, CBATCH * 128], DT.bfloat16, tag="xc")
                nc.scalar.dma_start(out=xc[:, :nn], in_=xT16[:, c0:c0 + nn])
                hrow = p1.tile([128, CBATCH * GE], DT.bfloat16, tag="hrow")
                for k in range(nb):
                    m = min(128, V - (c0 + k * 128))
                    hps = p1ps.tile([128, HEC], DT.float32, tag="hps")
                    nc.tensor.matmul(
                        out=hps[:m, :],
                        lhsT=xc[:, k * 128:k * 128 + m],
                        rhs=wext16[:],
                        start=True, stop=True)
                    nc.vector.tensor_copy(
                        out=hrow[:m, k * GE:k * GE + HEC],
                        in_=hps[:m, :])
                # contiguous full-row batched writes (junk cols included;
                # never read)
                last = min(V, c0 + nb * 128)
                kfull = (last - c0) // 128  # full 128-row chunks in batch
                if kfull > 0:
                    nc.sync.dma_start(
                        out=bass.AP(h_ext[:, :].tensor, c0 * GE,
                                    [[GE, 128], [GE * 128, kfull], [1, GE]]),
                        in_=hrow[:].rearrange("p (k c) -> p k c", c=GE)[
                            :, 0:kfull, :])
                for k in range(kfull, nb):
                    m = min(128, V - (c0 + k * 128))
                    nc.sync.dma_start(
                        out=h_ext[c0 + k * 128:c0 + k * 128 + m, :],
                        in_=hrow[:m, k * GE:(k + 1) * GE])

            # ---- phase 2 ----
            qs = [0, 1, 2, 3]
            for b in range(NBLK):
                rows = min(128, DEV_N - b * 128)
                bT8 = b * T * 8
                dl = p2.tile([128, T], DT.int16, tag="dl")
                nc.sync.dma_start(out=dl[:], in_=dstL[:, b * T:(b + 1) * T])
                gi = p2.tile([128, T * 8], DT.int16, tag="gi")
                nc.sync.dma_start(out=gi[:], in_=gidx[:, bT8:bT8 + T * 8])
                dlb = pdlb.tile([128, T * 128], DT.int16, tag="dlb")
                nc.sync.dma_start(
                    out=dlb[:],
                    in_=dstLb[:, b * T * 128:(b + 1) * T * 128])
                a_blk = p2.tile([128, 2 * NH], DT.bfloat16, tag="a_blk")
                nc.sync.dma_start(out=a_blk[:],
                                  in_=h_ext[b * 128:(b + 1) * 128, F:F + 2 * NH])

                stage = pb.tile([128, T * GE], DT.bfloat16, tag="stage")
                sr = stage[:].rearrange("p (t g) -> p t g", g=GE)
                nc.gpsimd.dma_gather(
                    out_ap=sr[:, 0:T_LO, :],
                    in_ap=h_ext[0:, :],
                    idxs_ap=gi[:, 0:T_LO * 8],
                    num_idxs=T_LO * 128, num_idxs_reg=T_LO * 128,
                    elem_size=GE, single_packet=False,
                    queue_num=qs[(2 * b) % 4])
                nc.gpsimd.dma_gather(
                    out_ap=sr[:, T_LO:T, :],
                    in_ap=h_ext[HALF:, :],
                    idxs_ap=gi[:, T_LO * 8:T * 8],
                    num_idxs=T_HI * 128, num_idxs_reg=T_HI * 128,
                    elem_size=GE, single_packet=False,
                    queue_num=qs[(2 * b + 1) % 4])

                # transposed one-hot selT[m, (t, e)] = (dstL[e, t] == m), bf16
                selT = prhs.tile([128, T * 128], DT.bfloat16, tag="selT")
                selTr = selT[:].rearrange("p (t e) -> p t e", e=128)
                nc.vector.tensor_tensor(
                    out=selTr[:, 0:T, :],
                    in0=dlb[:].rearrange("p (t e) -> p t e", e=128),
                    in1=iota_cw[:].rearrange("p (t e) -> p t e", e=128),
                    op=ALU.is_equal)
                # a_dst[e, h] per edge via one-hot matmul
                par = p2ps.tile([128, T * NH], DT.float32, tag="par")
                parr = par[:].rearrange("p (t e) -> p t e", e=NH)
                for j in range(T):
                    nc.tensor.matmul(
                        out=parr[:, j, :], lhsT=selTr[:, j, :],
                        rhs=a_blk[:, NH:2 * NH], start=True, stop=True)

                # one-hot sel[e, (t, m)] = (dstL[e, t] == m), bf16
                sel = prhs.tile([128, T * 128], DT.bfloat16, tag="sel")
                selr = sel[:].rearrange("p (t m) -> p t m", m=128)
                nc.vector.tensor_tensor(
                    out=selr[:, 0:T, :],
                    in0=dl[:][:, :, None].to_broadcast([128, T, 128]),
                    in1=iota16[:][:, None, :].to_broadcast([128, T, 128]),
                    op=ALU.is_equal)

                rhs = prhs.tile([128, T * RC], DT.bfloat16, tag="rhs")
                rr = rhs[:].rearrange("p (t c) -> p t c", c=RC)

                # ea chain: alpha -> leaky -> exp into rhs[:, :, 128:132]
                scr = p2.tile([128, T * NH], DT.float32, tag="scr")
                scrr = scr[:].rearrange("p (t e) -> p t e", e=NH)
                nc.vector.tensor_tensor(
                    out=scrr[:, 0:T, :], in0=sr[:, 0:T, F:F + NH],
                    in1=parr[:, 0:T, :], op=ALU.add)
                scr2 = p2.tile([128, T * NH], DT.float32, tag="scr2")
                scr2r = scr2[:].rearrange("p (t e) -> p t e", e=NH)
                nc.vector.tensor_tensor(
                    out=scr2r[:, 0:T, :], in0=scrr[:, 0:T, :],
                    in1=bass.AP(leak_c[:].tensor, 0,
                                [[1, 128], [0, T], [0, NH]]),
                    op=ALU.mult)
                nc.vector.tensor_tensor(
                    out=scrr[:, 0:T, :], in0=scrr[:, 0:T, :],
                    in1=scr2r[:, 0:T, :], op=ALU.max)
                nc.scalar.activation(out=rr[:, 0:T, F:F + NH],
                                     in_=scrr[:, 0:T, :], func=ACTF.Exp)

                # h copy on scalar engine
                nc.scalar.copy(out=rr[:, 0:T, F + NH:RC],
                               in_=sr[:, 0:T, 0:F])
                # Gs = h * ea (per-head broadcast)
                nc.vector.tensor_tensor(
                    out=rr[:, 0:T, 0:F].rearrange(
                        "p t (h e) -> p t h e", e=HD),
                    in0=sr[:, 0:T, 0:F].rearrange(
                        "p t (h e) -> p t h e", e=HD),
                    in1=rr[:, 0:T, F:F + NH][:, :, :, None].to_broadcast(
                        [128, T, NH, HD]),
                    op=ALU.mult)

                # accumulate
                acc = p2ps.tile([128, RC], DT.float32, tag="acc")
                for j in range(T):
                    nc.tensor.matmul(
                        out=acc[:], lhsT=selr[:, j, :], rhs=rr[:, j, :],
                        start=(j == 0), stop=(j == T - 1))

                # ---- evac: out = P / s + Q ----
                sden = p2.tile([128, NH], DT.float32, tag="sden")
                nc.vector.tensor_scalar_max(out=sden[:], in0=acc[:, F:F + NH],
                                            scalar1=1e-30)
                rs = p2.tile([128, NH], DT.float32, tag="rs")
                nc.vector.reciprocal(out=rs[:], in_=sden[:])
                ot = p2.tile([128, F], DT.float32, tag="ot")
                otr = ot[:].rearrange("p (h e) -> p h e", e=HD)
                nc.vector.tensor_tensor(
                    out=otr,
                    in0=acc[:, 0:F].rearrange("p (h e) -> p h e", e=HD),
                    in1=rs[:][:, :, None].to_broadcast([128, NH, HD]),
                    op=ALU.mult)
                nc.vector.tensor_tensor(
                    out=otr, in0=otr,
                    in1=acc[:, F + NH:RC].rearrange("p (h e) -> p h e", e=HD),
                    op=ALU.add)
                nc.sync.dma_start(out=out[b * 128:b * 128 + rows, :],
                                  in_=ot[:rows, :])

    return nc


def route_edges(edge_index, N, n_cores, half=32768):
    """Host edge routing. Returns (T_LO, T_HI, per_core index dicts)."""
    src = np.concatenate([np.asarray(edge_index[0]),
                          np.arange(N)]).astype(np.int64)
    dst = np.concatenate([np.asarray(edge_index[1]),
                          np.arange(N)]).astype(np.int64)
    dev_n = N // n_cores
    assert dev_n * n_cores == N
    core = dst // dev_n
    nblk = (dev_n + 127) // 128

    per_core_raw = []
    T_LO = T_HI = 0
    for d in range(n_cores):
        m = core == d
        s_rot = (src[m] - d * dev_n) % N
        d_loc = dst[m] - d * dev_n
        blk = d_loc // 128
        lo = s_rot < half
        cnt_lo = np.bincount(blk[lo], minlength=nblk)
        cnt_hi = np.bincount(blk[~lo], minlength=nblk)
        T_LO = max(T_LO, int(-(-cnt_lo.max() // 128)))
        T_HI = max(T_HI, int(-(-cnt_hi.max() // 128)))
        per_core_raw.append((s_rot, d_loc, blk, lo))
    T_HI = max(T_HI, 1)
    T_LO = max(T_LO, 1)
    T = T_LO + T_HI

    per_core = []
    for d in range(n_cores):
        s_rot, d_loc, blk, lo = per_core_raw[d]
        gidx16 = np.zeros((16, nblk * T * 8), dtype=np.int16)
        dstL = np.full((128, nblk * T), -1, dtype=np.int16)
        for b in range(nblk):
            bcol = b * T * 8
            for sec in (0, 1):
                if sec == 0:
                    bm = (blk == b) & lo
                    voff, t0, sec_col = 0, 0, bcol
                else:
                    bm = (blk == b) & ~lo
                    voff, t0, sec_col = half, T_LO, bcol + T_LO * 8
                vals = s_rot[bm]
                dloc_b = d_loc[bm]
                n = len(vals)
                if n == 0:
                    continue
                order = np.argsort(vals, kind="stable")
                vals = vals[order] - voff
                dloc_b = dloc_b[order]
                jj = np.arange(n)
                gidx16[jj % 16, sec_col + jj // 16] = vals.astype(np.int16)
                dstL[jj % 128, b * T + t0 + jj // 128] = (
                    dloc_b - b * 128).astype(np.int16)

        # dstLb[m, (b, t, e)] = dstL[e, b*T + t], replicated down partitions
        dstLb = np.ascontiguousarray(np.broadcast_to(
            dstL.T.reshape(1, -1), (128, nblk * T * 128)).astype(np.int16))
        per_core.append({
            "gidx": np.tile(gidx16, (8, 1)),
            "dstLb": dstLb,
            "dstL": dstL,
        })
    return T_LO, T_HI, per_core


def host_prep(x, edge_index, W, att_src, att_dst, n_cores, half=32768):
    """Returns (T_LO, T_HI, per-core in_maps list)."""
    N = x.shape[0]
    dev_n = N // n_cores
    bf16 = DT.np(DT.bfloat16)
    xTf = np.ascontiguousarray(np.asarray(x).T.astype(np.float32))
    Wnat = np.ascontiguousarray(np.asarray(W).astype(np.float32))
    Wt = np.ascontiguousarray(Wnat.T)
    A = np.zeros((F, 2 * NH), dtype=np.float32)
    for h in range(NH):
        A[h * HD:(h + 1) * HD, h] = np.asarray(att_src)[0, h]
        A[h * HD:(h + 1) * HD, NH + h] = np.asarray(att_dst)[0, h]
    T_LO, T_HI, per_core = route_edges(edge_index, N, n_cores, half)
    in_maps = []
    for d in range(n_cores):
        xr = np.roll(xTf, -d * dev_n, axis=1)
        in_maps.append(dict(per_core[d],
                            xT16=np.ascontiguousarray(xr.astype(bf16)),
                            Wnat=Wnat, Wt=Wt, Aatt=A))
    return T_LO, T_HI, in_maps


# ---------------------------------------------------------------------------
# Self-contained kernel entry point (full problem size hardcoded).
# ---------------------------------------------------------------------------
N_NODES = 50000
N_CORES = 8
HALF_SPLIT = 32768


def _run(inputs, trace=False):
    import time
    from concourse.bass_utils import run_bass_kernel_spmd

    global LAST_RES
    x = np.asarray(inputs["x"], dtype=np.float32)
    edge_index = np.asarray(inputs["edge_index"])
    W = np.asarray(inputs["W"], dtype=np.float32)
    att_src = np.asarray(inputs["att_src"], dtype=np.float32)
    att_dst = np.asarray(inputs["att_dst"], dtype=np.float32)

    N = x.shape[0]
    assert N == N_NODES, N
    dev_n = N // N_CORES

    t0 = time.time()
    T_LO, T_HI, in_maps = host_prep(x, edge_index, W, att_src, att_dst,
                                    N_CORES, half=HALF_SPLIT)
    t1 = time.time()
    nc = build_gat_nc(N, dev_n, T_LO, T_HI, HALF=HALF_SPLIT)
    nc.compile()
    t2 = time.time()
    res = run_bass_kernel_spmd(nc, in_maps, list(range(N_CORES)), trace=trace)
    LAST_RES = res
    t3 = time.time()
    print(f"kernel: host_prep {t1-t0:.1f}s build+compile {t2-t1:.1f}s "
          f"run {t3-t2:.1f}s T_LO={T_LO} T_HI={T_HI}")
    out = np.concatenate([res.results[d]["out"] for d in range(N_CORES)],
                         axis=0).astype(np.float32)
    return out, res.exec_time_ns


def kernel(**inputs) -> np.ndarray:
    return _run(inputs, trace=False)[0]


# revision 38
# speedup vs baseline: 1.1662x; 1.0091x over previous
"""GATConv Trainium kernel (single-core SPMD program) + host prep.  V2.

Per-core program (identical NEFF on all 8 cores, different input data):
  Node tables are ROTATED per core: table row r = global node
  (dev_base + r) % N, so every core's own nodes are rows 0..DEV_N-1 and the
  program stays core-independent. The host rotates xT and all indices.

  Phase 1 (all V rows): one packed bf16 table
  h_ext[r, 0:136] = [h = x@W.T (128) | a_src (4) | a_dst (4)], 256-wide bf16
  rows (512B, dma_gather elem multiple of 256B; cols 136:256 unwritten junk,
  never read).

  Phase 2, per dst-block (128 own nodes), edges pre-routed/sorted by host:
  - h-gather: full 512B rows of h_ext by src (int16 idx; lo section src <
    32768 from h_ext[0:], hi section src-32768 from h_ext[32768:]) -> stage
    [e, t, 256]: h at 0:128, a_src at 128:132.
  - a-gather: 256B half-rows h_ext[:, 128:256] by dst row (= local dst,
    rows 0..DEV_N-1, single section) -> astage [e, t, 128]: a_dst at 4:8.
  - ea = exp(leaky(a_src[src] + a_dst[dst])) (bf16), Gs = h[src]*ea.
  - rhs[e, t, 0:260] = [Gs(128) | ea(4) | h(128)]; one-hot
    sel[e, m] = (dst_loc[e] == m) in bf16; PSUM acc accumulates
    sel.T @ rhs over the block's T tiles => [P | s | Q].
  - out = P/s + Q.

Edge layout: per block, lo-section edges then hi-section edges, sorted by
src within each section (DRAM row locality for the gather), each padded to
global fixed tile counts (T_LO / T_HI) with idx-0 edges carrying
dst_loc = -1 (zero one-hot row => no contribution). Edge i of a section is
at (lane = i%128, tile = i//128); dma_gather's index j lives at
idx16[j%16, j//16], replicated 8x down the 128 partitions.
"""

import numpy as np

import concourse.bass as bass
import concourse.bacc as bacc
import concourse.mybir as mybir
import concourse.tile as tile
from concourse import library_config

DT = mybir.dt
ALU = mybir.AluOpType
ACTF = mybir.ActivationFunctionType

F = 128    # feature dim (in == out)
NH = 4     # heads
HD = 32    # head dim = 32
HEC = 136  # used h_ext cols: h(128) | a_src(4) | a_dst(4)
GE = 256   # h_ext row elems (bf16 -> 512B, mult of 256B)
AE = 128   # a-gather elem width (bf16 -> 256B)
RC = 260   # rhs per-tile block: Gs(128) | ea(4) | h(128)


def build_gat_nc(V, DEV_N, T_LO, T_HI, HALF=32768, leaky=0.2):
    """Build the single-core Bass program."""
    T = T_LO + T_HI
    NBLK = (DEV_N + 127) // 128

    nc = bacc.Bacc(num_swdge_queues=4)
    xT16 = nc.declare_dram_parameter("xT16", [F, V], DT.bfloat16,
                                     isOutput=False)
    Wnat = nc.declare_dram_parameter("Wnat", [F, F], DT.float32,
                                     isOutput=False)
    Wt = nc.declare_dram_parameter("Wt", [F, F], DT.float32, isOutput=False)
    Aatt = nc.declare_dram_parameter("Aatt", [F, 2 * NH], DT.float32,
                                     isOutput=False)
    gidx = nc.declare_dram_parameter("gidx", [128, NBLK * T * 8], DT.int16,
                                     isOutput=False)
    dstLb = nc.declare_dram_parameter("dstLb", [128, NBLK * T * 128],
                                      DT.int16, isOutput=False)
    dstL = nc.declare_dram_parameter("dstL", [128, NBLK * T], DT.int16,
                                     isOutput=False)
    out = nc.declare_dram_parameter("out", [DEV_N, F], DT.float32,
                                    isOutput=True)

    h_ext = nc.dram_tensor("h_ext", [V, GE], DT.bfloat16)

    with tile.TileContext(nc) as tc:
        with (
            tc.tile_pool(name="const", bufs=1) as const,
            tc.tile_pool(name="p1", bufs=3) as p1,
            tc.tile_pool(name="p1ps", bufs=2, space="PSUM") as p1ps,
            tc.tile_pool(name="p2", bufs=4) as p2,
            tc.tile_pool(name="pdlb", bufs=2) as pdlb,
            tc.tile_pool(name="pb", bufs=3) as pb,
            tc.tile_pool(name="prhs", bufs=2) as prhs,
            tc.tile_pool(name="p2ps", bufs=2, space="PSUM") as p2ps,
        ):
            nc.gpsimd.load_library(library_config.mlp)

            # ---- constants ----
            wnat_t = const.tile([128, F], DT.float32)
            aatt_t = const.tile([128, 2 * NH], DT.float32)
            wt_t = const.tile([128, F], DT.float32)
            wext16 = const.tile([128, HEC], DT.bfloat16)
            iota32 = const.tile([128, 128], DT.int32)
            iota16 = const.tile([128, 128], DT.int16)
            iotac32 = const.tile([128, 1], DT.int32)
            iotac16 = const.tile([128, 1], DT.int16)
            # per-partition index replicated along free dim, for selT build
            iota_cw = const.tile([128, T * 128], DT.int16)
            leak_c = const.tile([128, 1], DT.float32)
            nc.gpsimd.memset(leak_c[:], leaky)
            nc.sync.dma_start(out=wnat_t[:], in_=Wnat[:, :])
            nc.sync.dma_start(out=aatt_t[:], in_=Aatt[:, :])
            nc.sync.dma_start(out=wt_t[:], in_=Wt[:, :])
            nc.gpsimd.iota(iota32[:], pattern=[[1, 128]], base=0,
                           channel_multiplier=0)
            nc.vector.tensor_copy(out=iota16[:], in_=iota32[:])
            nc.gpsimd.iota(iotac32[:], pattern=[[0, 1]], base=0,
                           channel_multiplier=1)
            nc.vector.tensor_copy(out=iotac16[:], in_=iotac32[:])
            nc.vector.tensor_copy(
                out=iota_cw[:],
                in_=bass.AP(iotac16[:].tensor, 0, [[1, 128], [0, T * 128]]))
            vps = p1ps.tile([128, 2 * NH], DT.float32, tag="vps")
            nc.tensor.matmul(out=vps[:], lhsT=wnat_t[:], rhs=aatt_t[:],
                             start=True, stop=True)
            nc.vector.tensor_copy(out=wext16[:, 0:F], in_=wt_t[:])
            nc.vector.tensor_copy(out=wext16[:, F:HEC], in_=vps[:])

            # ---- phase 1 (batches of 8 node chunks) ----
            nchunks = (V + 127) // 128
            CBATCH = 8
            for cb in range(0, nchunks, CBATCH):
                nb = min(CBATCH, nchunks - cb)
                c0 = cb * 128
                nn = min(V - c0, nb * 128)
                xc = p1.tile([128, CBATCH * 128], DT.bfloat16, tag="xc")
                nc.scalar.dma_start(out=xc[:, :nn], in_=xT16[:, c0:c0 + nn])
                hrow = p1.tile([128, CBATCH * GE], DT.bfloat16, tag="hrow")
                for k in range(nb):
                    m = min(128, V - (c0 + k * 128))
                    hps = p1ps.tile([128, HEC], DT.float32, tag="hps")
                    nc.tensor.matmul(
                        out=hps[:m, :],
                        lhsT=xc[:, k * 128:k * 128 + m],
                        rhs=wext16[:],
                        start=True, stop=True)
                    nc.vector.tensor_copy(
                        out=hrow[:m, k * GE:k * GE + HEC],
                        in_=hps[:m, :])
                # contiguous full-row batched writes (junk cols included;
                # never read)
                last = min(V, c0 + nb * 128)
                kfull = (last - c0) // 128  # full 128-row chunks in batch
                if kfull > 0:
                    nc.sync.dma_start(
                        out=bass.AP(h_ext[:, :].tensor, c0 * GE,
                                    [[GE, 128], [GE * 128, kfull], [1, GE]]),
                        in_=hrow[:].rearrange("p (k c) -> p k c", c=GE)[
                            :, 0:kfull, :])
                for k in range(kfull, nb):
                    m = min(128, V - (c0 + k * 128))
                    nc.sync.dma_start(
                        out=h_ext[c0 + k * 128:c0 + k * 128 + m, :],
                        in_=hrow[:m, k * GE:(k + 1) * GE])

            # ---- phase 2 ----
            qs = [0, 1, 2, 3]
            for b in range(NBLK):
                rows = min(128, DEV_N - b * 128)
                bT8 = b * T * 8
                dl = p2.tile([128, T], DT.int16, tag="dl")
                nc.sync.dma_start(out=dl[:], in_=dstL[:, b * T:(b + 1) * T])
                gi = p2.tile([128, T * 8], DT.int16, tag="gi")
                nc.sync.dma_start(out=gi[:], in_=gidx[:, bT8:bT8 + T * 8])
                dlb = pdlb.tile([128, T * 128], DT.int16, tag="dlb")
                nc.sync.dma_start(
                    out=dlb[:],
                    in_=dstLb[:, b * T * 128:(b + 1) * T * 128])
                a_blk = p2.tile([128, 2 * NH], DT.bfloat16, tag="a_blk")
                nc.sync.dma_start(out=a_blk[:],
                                  in_=h_ext[b * 128:(b + 1) * 128, F:F + 2 * NH])

                # separate lo/hi stage tiles: disjoint tiles let the two
                # gathers run concurrently (whole-tile WAW tracking)
                stage_lo = pb.tile([128, T_LO * GE], DT.bfloat16,
                                   tag="stage_lo")
                srl = stage_lo[:].rearrange("p (t g) -> p t g", g=GE)
                stage_hi = pb.tile([128, T_HI * GE], DT.bfloat16,
                                   tag="stage_hi")
                srh = stage_hi[:].rearrange("p (t g) -> p t g", g=GE)
                nc.gpsimd.dma_gather(
                    out_ap=srl[:, 0:T_LO, :],
                    in_ap=h_ext[0:, :],
                    idxs_ap=gi[:, 0:T_LO * 8],
                    num_idxs=T_LO * 128, num_idxs_reg=T_LO * 128,
                    elem_size=GE, single_packet=False,
                    queue_num=qs[(2 * b) % 4])
                nc.gpsimd.dma_gather(
                    out_ap=srh[:, 0:T_HI, :],
                    in_ap=h_ext[HALF:, :],
                    idxs_ap=gi[:, T_LO * 8:T * 8],
                    num_idxs=T_HI * 128, num_idxs_reg=T_HI * 128,
                    elem_size=GE, single_packet=False,
                    queue_num=qs[(2 * b + 1) % 4])

                # transposed one-hot selT[m, (t, e)] = (dstL[e, t] == m), bf16
                selT = prhs.tile([128, T * 128], DT.bfloat16, tag="selT")
                selTr = selT[:].rearrange("p (t e) -> p t e", e=128)
                nc.vector.tensor_tensor(
                    out=selTr[:, 0:T, :],
                    in0=dlb[:].rearrange("p (t e) -> p t e", e=128),
                    in1=iota_cw[:].rearrange("p (t e) -> p t e", e=128),
                    op=ALU.is_equal)
                # a_dst[e, h] per edge via one-hot matmul
                par = p2ps.tile([128, T * NH], DT.float32, tag="par")
                parr = par[:].rearrange("p (t e) -> p t e", e=NH)
                for j in range(T):
                    nc.tensor.matmul(
                        out=parr[:, j, :], lhsT=selTr[:, j, :],
                        rhs=a_blk[:, NH:2 * NH], start=True, stop=True)

                # one-hot sel[e, (t, m)] = (dstL[e, t] == m), bf16
                sel = prhs.tile([128, T * 128], DT.bfloat16, tag="sel")
                selr = sel[:].rearrange("p (t m) -> p t m", m=128)
                nc.vector.tensor_tensor(
                    out=selr[:, 0:T, :],
                    in0=dl[:][:, :, None].to_broadcast([128, T, 128]),
                    in1=iota16[:][:, None, :].to_broadcast([128, T, 128]),
                    op=ALU.is_equal)

                rhs = prhs.tile([128, T * RC], DT.bfloat16, tag="rhs")
                rr = rhs[:].rearrange("p (t c) -> p t c", c=RC)

                # ea chain: alpha -> leaky -> exp into rhs[:, :, 128:132]
                scr = p2.tile([128, T * NH], DT.float32, tag="scr")
                scrr = scr[:].rearrange("p (t e) -> p t e", e=NH)
                nc.vector.tensor_tensor(
                    out=scrr[:, 0:T_LO, :], in0=srl[:, 0:T_LO, F:F + NH],
                    in1=parr[:, 0:T_LO, :], op=ALU.add)
                nc.vector.tensor_tensor(
                    out=scrr[:, T_LO:T, :], in0=srh[:, 0:T_HI, F:F + NH],
                    in1=parr[:, T_LO:T, :], op=ALU.add)
                scr2 = p2.tile([128, T * NH], DT.float32, tag="scr2")
                scr2r = scr2[:].rearrange("p (t e) -> p t e", e=NH)
                nc.vector.tensor_tensor(
                    out=scr2r[:, 0:T, :], in0=scrr[:, 0:T, :],
                    in1=bass.AP(leak_c[:].tensor, 0,
                                [[1, 128], [0, T], [0, NH]]),
                    op=ALU.mult)
                nc.vector.tensor_tensor(
                    out=scrr[:, 0:T, :], in0=scrr[:, 0:T, :],
                    in1=scr2r[:, 0:T, :], op=ALU.max)
                nc.scalar.activation(out=rr[:, 0:T, F:F + NH],
                                     in_=scrr[:, 0:T, :], func=ACTF.Exp)

                # h copy on scalar engine
                nc.scalar.copy(out=rr[:, 0:T_LO, F + NH:RC],
                               in_=srl[:, 0:T_LO, 0:F])
                nc.scalar.copy(out=rr[:, T_LO:T, F + NH:RC],
                               in_=srh[:, 0:T_HI, 0:F])
                # Gs = h * ea (per-head broadcast)
                nc.vector.tensor_tensor(
                    out=rr[:, 0:T_LO, 0:F].rearrange(
                        "p t (h e) -> p t h e", e=HD),
                    in0=srl[:, 0:T_LO, 0:F].rearrange(
                        "p t (h e) -> p t h e", e=HD),
                    in1=rr[:, 0:T_LO, F:F + NH][:, :, :, None].to_broadcast(
                        [128, T_LO, NH, HD]),
                    op=ALU.mult)
                nc.vector.tensor_tensor(
                    out=rr[:, T_LO:T, 0:F].rearrange(
                        "p t (h e) -> p t h e", e=HD),
                    in0=srh[:, 0:T_HI, 0:F].rearrange(
                        "p t (h e) -> p t h e", e=HD),
                    in1=rr[:, T_LO:T, F:F + NH][:, :, :, None].to_broadcast(
                        [128, T_HI, NH, HD]),
                    op=ALU.mult)

                # accumulate
                acc = p2ps.tile([128, RC], DT.float32, tag="acc")
                for j in range(T):
                    nc.tensor.matmul(
                        out=acc[:], lhsT=selr[:, j, :], rhs=rr[:, j, :],
                        start=(j == 0), stop=(j == T - 1))

                # ---- evac: out = P / s + Q ----
                sden = p2.tile([128, NH], DT.float32, tag="sden")
                nc.vector.tensor_scalar_max(out=sden[:], in0=acc[:, F:F + NH],
                                            scalar1=1e-30)
                rs = p2.tile([128, NH], DT.float32, tag="rs")
                nc.vector.reciprocal(out=rs[:], in_=sden[:])
                ot = p2.tile([128, F], DT.float32, tag="ot")
                otr = ot[:].rearrange("p (h e) -> p h e", e=HD)
                nc.vector.tensor_tensor(
                    out=otr,
                    in0=acc[:, 0:F].rearrange("p (h e) -> p h e", e=HD),
                    in1=rs[:][:, :, None].to_broadcast([128, NH, HD]),
                    op=ALU.mult)
                nc.vector.tensor_tensor(
                    out=otr, in0=otr,
                    in1=acc[:, F + NH:RC].rearrange("p (h e) -> p h e", e=HD),
                    op=ALU.add)
                nc.sync.dma_start(out=out[b * 128:b * 128 + rows, :],
                                  in_=ot[:rows, :])

    return nc


def route_edges(edge_index, N, n_cores, half=32768):
    """Host edge routing. Returns (T_LO, T_HI, per_core index dicts)."""
    src = np.concatenate([np.asarray(edge_index[0]),
                          np.arange(N)]).astype(np.int64)
    dst = np.concatenate([np.asarray(edge_index[1]),
                          np.arange(N)]).astype(np.int64)
    dev_n = N // n_cores
    assert dev_n * n_cores == N
    core = dst // dev_n
    nblk = (dev_n + 127) // 128

    per_core_raw = []
    T_LO = T_HI = 0
    for d in range(n_cores):
        m = core == d
        s_rot = (src[m] - d * dev_n) % N
        d_loc = dst[m] - d * dev_n
        blk = d_loc // 128
        lo = s_rot < half
        cnt_lo = np.bincount(blk[lo], minlength=nblk)
        cnt_hi = np.bincount(blk[~lo], minlength=nblk)
        T_LO = max(T_LO, int(-(-cnt_lo.max() // 128)))
        T_HI = max(T_HI, int(-(-cnt_hi.max() // 128)))
        per_core_raw.append((s_rot, d_loc, blk, lo))
    T_HI = max(T_HI, 1)
    T_LO = max(T_LO, 1)
    T = T_LO + T_HI

    per_core = []
    for d in range(n_cores):
        s_rot, d_loc, blk, lo = per_core_raw[d]
        gidx16 = np.zeros((16, nblk * T * 8), dtype=np.int16)
        dstL = np.full((128, nblk * T), -1, dtype=np.int16)
        for b in range(nblk):
            bcol = b * T * 8
            for sec in (0, 1):
                if sec == 0:
                    bm = (blk == b) & lo
                    voff, t0, sec_col = 0, 0, bcol
                else:
                    bm = (blk == b) & ~lo
                    voff, t0, sec_col = half, T_LO, bcol + T_LO * 8
                vals = s_rot[bm]
                dloc_b = d_loc[bm]
                n = len(vals)
                if n == 0:
                    continue
                order = np.argsort(vals, kind="stable")
                vals = vals[order] - voff
                dloc_b = dloc_b[order]
                jj = np.arange(n)
                gidx16[jj % 16, sec_col + jj // 16] = vals.astype(np.int16)
                dstL[jj % 128, b * T + t0 + jj // 128] = (
                    dloc_b - b * 128).astype(np.int16)

        # dstLb[m, (b, t, e)] = dstL[e, b*T + t], replicated down partitions
        dstLb = np.ascontiguousarray(np.broadcast_to(
            dstL.T.reshape(1, -1), (128, nblk * T * 128)).astype(np.int16))
        per_core.append({
            "gidx": np.tile(gidx16, (8, 1)),
            "dstLb": dstLb,
            "dstL": dstL,
        })
    return T_LO, T_HI, per_core


def host_prep(x, edge_index, W, att_src, att_dst, n_cores, half=32768):
    """Returns (T_LO, T_HI, per-core in_maps list)."""
    N = x.shape[0]
    dev_n = N // n_cores
    bf16 = DT.np(DT.bfloat16)
    xTf = np.ascontiguousarray(np.asarray(x).T.astype(np.float32))
    Wnat = np.ascontiguousarray(np.asarray(W).astype(np.float32))
    Wt = np.ascontiguousarray(Wnat.T)
    A = np.zeros((F, 2 * NH), dtype=np.float32)
    for h in range(NH):
        A[h * HD:(h + 1) * HD, h] = np.asarray(att_src)[0, h]
        A[h * HD:(h + 1) * HD, NH + h] = np.asarray(att_dst)[0, h]
    T_LO, T_HI, per_core = route_edges(edge_index, N, n_cores, half)
    in_maps = []
    for d in range(n_cores):
        xr = np.roll(xTf, -d * dev_n, axis=1)
        in_maps.append(dict(per_core[d],
                            xT16=np.ascontiguousarray(xr.astype(bf16)),
                            Wnat=Wnat, Wt=Wt, Aatt=A))
    return T_LO, T_HI, in_maps


# ---------------------------------------------------------------------------
# Self-contained kernel entry point (full problem size hardcoded).
# ---------------------------------------------------------------------------
N_NODES = 50000
N_CORES = 8
HALF_SPLIT = 32768


def _run(inputs, trace=False):
    import time
    from concourse.bass_utils import run_bass_kernel_spmd

    global LAST_RES
    x = np.asarray(inputs["x"], dtype=np.float32)
    edge_index = np.asarray(inputs["edge_index"])
    W = np.asarray(inputs["W"], dtype=np.float32)
    att_src = np.asarray(inputs["att_src"], dtype=np.float32)
    att_dst = np.asarray(inputs["att_dst"], dtype=np.float32)

    N = x.shape[0]
    assert N == N_NODES, N
    dev_n = N // N_CORES

    t0 = time.time()
    T_LO, T_HI, in_maps = host_prep(x, edge_index, W, att_src, att_dst,
                                    N_CORES, half=HALF_SPLIT)
    t1 = time.time()
    nc = build_gat_nc(N, dev_n, T_LO, T_HI, HALF=HALF_SPLIT)
    nc.compile()
    t2 = time.time()
    res = run_bass_kernel_spmd(nc, in_maps, list(range(N_CORES)), trace=trace)
    LAST_RES = res
    t3 = time.time()
    print(f"kernel: host_prep {t1-t0:.1f}s build+compile {t2-t1:.1f}s "
          f"run {t3-t2:.1f}s T_LO={T_LO} T_HI={T_HI}")
    out = np.concatenate([res.results[d]["out"] for d in range(N_CORES)],
                         axis=0).astype(np.float32)
    return out, res.exec_time_ns


def kernel(**inputs) -> np.ndarray:
    return _run(inputs, trace=False)[0]


# revision 41
# speedup vs baseline: 1.4327x; 1.2285x over previous
"""GATConv Trainium kernel (single-core SPMD program) + host prep.  V2.

Per-core program (identical NEFF on all 8 cores, different input data):
  Node tables are ROTATED per core: table row r = global node
  (dev_base + r) % N, so every core's own nodes are rows 0..DEV_N-1 and the
  program stays core-independent. The host rotates xT and all indices.

  Phase 1 (all V rows): one packed bf16 table
  h_ext[r, 0:136] = [h = x@W.T (128) | a_src (4) | a_dst (4)], 256-wide bf16
  rows (512B, dma_gather elem multiple of 256B; cols 136:256 unwritten junk,
  never read).

  Phase 2, per dst-block (128 own nodes), edges pre-routed/sorted by host:
  - h-gather: full 512B rows of h_ext by src (int16 idx; lo section src <
    32768 from h_ext[0:], hi section src-32768 from h_ext[32768:]) -> stage
    [e, t, 256]: h at 0:128, a_src at 128:132.
  - a-gather: 256B half-rows h_ext[:, 128:256] by dst row (= local dst,
    rows 0..DEV_N-1, single section) -> astage [e, t, 128]: a_dst at 4:8.
  - ea = exp(leaky(a_src[src] + a_dst[dst])) (bf16), Gs = h[src]*ea.
  - rhs[e, t, 0:260] = [Gs(128) | ea(4) | h(128)]; one-hot
    sel[e, m] = (dst_loc[e] == m) in bf16; PSUM acc accumulates
    sel.T @ rhs over the block's T tiles => [P | s | Q].
  - out = P/s + Q.

Edge layout: per block, lo-section edges then hi-section edges, sorted by
src within each section (DRAM row locality for the gather), each padded to
global fixed tile counts (T_LO / T_HI) with idx-0 edges carrying
dst_loc = -1 (zero one-hot row => no contribution). Edge i of a section is
at (lane = i%128, tile = i//128); dma_gather's index j lives at
idx16[j%16, j//16], replicated 8x down the 128 partitions.
"""

import numpy as np

import concourse.bass as bass
import concourse.bacc as bacc
import concourse.mybir as mybir
import concourse.tile as tile
from concourse import library_config

DT = mybir.dt
ALU = mybir.AluOpType
ACTF = mybir.ActivationFunctionType

F = 128    # feature dim (in == out)
NH = 4     # heads
HD = 32    # head dim = 32
HEC = 136  # used h_ext cols: h(128) | a_src(4) | a_dst(4)
GE = 256   # h_ext row elems (bf16 -> 512B, mult of 256B)
AE = 128   # a-gather elem width (bf16 -> 256B)
RC = 260   # rhs per-tile block: Gs(128) | ea(4) | h(128)


def build_gat_nc(V, DEV_N, T_LO, T_HI, HALF=32768, leaky=0.2):
    """Build the single-core Bass program."""
    T = T_LO + T_HI
    NBLK = (DEV_N + 127) // 128

    nc = bacc.Bacc(num_swdge_queues=4)
    xT16 = nc.declare_dram_parameter("xT16", [F, V], DT.bfloat16,
                                     isOutput=False)
    Wnat = nc.declare_dram_parameter("Wnat", [F, F], DT.float32,
                                     isOutput=False)
    Wt = nc.declare_dram_parameter("Wt", [F, F], DT.float32, isOutput=False)
    Aatt = nc.declare_dram_parameter("Aatt", [F, 2 * NH], DT.float32,
                                     isOutput=False)
    gidx = nc.declare_dram_parameter("gidx", [128, NBLK * T * 8], DT.int16,
                                     isOutput=False)
    dstLb = nc.declare_dram_parameter("dstLb", [128, NBLK * T * 128],
                                      DT.int16, isOutput=False)
    dstL = nc.declare_dram_parameter("dstL", [128, NBLK * T], DT.int16,
                                     isOutput=False)
    out = nc.declare_dram_parameter("out", [DEV_N, F], DT.float32,
                                    isOutput=True)

    h_ext = nc.dram_tensor("h_ext", [V, GE], DT.bfloat16)

    with tile.TileContext(nc) as tc:
        with (
            tc.tile_pool(name="const", bufs=1) as const,
            tc.tile_pool(name="p1", bufs=3) as p1,
            tc.tile_pool(name="p1ps", bufs=2, space="PSUM") as p1ps,
            tc.tile_pool(name="p2", bufs=4) as p2,
            tc.tile_pool(name="pdlb", bufs=2) as pdlb,
            tc.tile_pool(name="pb", bufs=3) as pb,
            tc.tile_pool(name="prhs", bufs=2) as prhs,
            tc.tile_pool(name="p2ps", bufs=2, space="PSUM") as p2ps,
        ):
            nc.gpsimd.load_library(library_config.mlp)

            # ---- constants ----
            wnat_t = const.tile([128, F], DT.float32)
            aatt_t = const.tile([128, 2 * NH], DT.float32)
            wt_t = const.tile([128, F], DT.float32)
            wext16 = const.tile([128, HEC], DT.bfloat16)
            iota32 = const.tile([128, 128], DT.int32)
            iota16 = const.tile([128, 128], DT.int16)
            iotac32 = const.tile([128, 1], DT.int32)
            iotac16 = const.tile([128, 1], DT.int16)
            # per-partition index replicated along free dim, for selT build
            iota_cw = const.tile([128, T * 128], DT.int16)
            leak_c = const.tile([128, 1], DT.float32)
            nc.gpsimd.memset(leak_c[:], leaky)
            nc.sync.dma_start(out=wnat_t[:], in_=Wnat[:, :])
            nc.sync.dma_start(out=aatt_t[:], in_=Aatt[:, :])
            nc.sync.dma_start(out=wt_t[:], in_=Wt[:, :])
            nc.gpsimd.iota(iota32[:], pattern=[[1, 128]], base=0,
                           channel_multiplier=0)
            nc.vector.tensor_copy(out=iota16[:], in_=iota32[:])
            nc.gpsimd.iota(iotac32[:], pattern=[[0, 1]], base=0,
                           channel_multiplier=1)
            nc.vector.tensor_copy(out=iotac16[:], in_=iotac32[:])
            nc.vector.tensor_copy(
                out=iota_cw[:],
                in_=bass.AP(iotac16[:].tensor, 0, [[1, 128], [0, T * 128]]))
            vps = p1ps.tile([128, 2 * NH], DT.float32, tag="vps")
            nc.tensor.matmul(out=vps[:], lhsT=wnat_t[:], rhs=aatt_t[:],
                             start=True, stop=True)
            nc.vector.tensor_copy(out=wext16[:, 0:F], in_=wt_t[:])
            nc.vector.tensor_copy(out=wext16[:, F:HEC], in_=vps[:])

            # ---- phase 1 (batches of 8 node chunks) ----
            nchunks = (V + 127) // 128
            CBATCH = 8
            for cb in range(0, nchunks, CBATCH):
                nb = min(CBATCH, nchunks - cb)
                c0 = cb * 128
                nn = min(V - c0, nb * 128)
                xc = p1.tile([128, CBATCH * 128], DT.bfloat16, tag="xc")
                nc.scalar.dma_start(out=xc[:, :nn], in_=xT16[:, c0:c0 + nn])
                hrow = p1.tile([128, CBATCH * GE], DT.bfloat16, tag="hrow")
                for k in range(nb):
                    m = min(128, V - (c0 + k * 128))
                    hps = p1ps.tile([128, HEC], DT.float32, tag="hps")
                    nc.tensor.matmul(
                        out=hps[:m, :],
                        lhsT=xc[:, k * 128:k * 128 + m],
                        rhs=wext16[:],
                        start=True, stop=True)
                    nc.vector.tensor_copy(
                        out=hrow[:m, k * GE:k * GE + HEC],
                        in_=hps[:m, :])
                # contiguous full-row batched writes (junk cols included;
                # never read)
                last = min(V, c0 + nb * 128)
                kfull = (last - c0) // 128  # full 128-row chunks in batch
                if kfull > 0:
                    nc.sync.dma_start(
                        out=bass.AP(h_ext[:, :].tensor, c0 * GE,
                                    [[GE, 128], [GE * 128, kfull], [1, GE]]),
                        in_=hrow[:].rearrange("p (k c) -> p k c", c=GE)[
                            :, 0:kfull, :])
                for k in range(kfull, nb):
                    m = min(128, V - (c0 + k * 128))
                    nc.sync.dma_start(
                        out=h_ext[c0 + k * 128:c0 + k * 128 + m, :],
                        in_=hrow[:m, k * GE:(k + 1) * GE])

            # ---- phase 2 ----
            qs = [0, 1, 2, 3]
            for b in range(NBLK):
                rows = min(128, DEV_N - b * 128)
                bT8 = b * T * 8
                dl = p2.tile([128, T], DT.int16, tag="dl")
                nc.sync.dma_start(out=dl[:], in_=dstL[:, b * T:(b + 1) * T])
                gi = p2.tile([128, T * 8], DT.int16, tag="gi")
                nc.sync.dma_start(out=gi[:], in_=gidx[:, bT8:bT8 + T * 8])
                dlb = pdlb.tile([128, T * 128], DT.int16, tag="dlb")
                nc.sync.dma_start(
                    out=dlb[:],
                    in_=dstLb[:, b * T * 128:(b + 1) * T * 128])
                a_blk = p2.tile([128, 2 * NH], DT.bfloat16, tag="a_blk")
                nc.sync.dma_start(out=a_blk[:],
                                  in_=h_ext[b * 128:(b + 1) * 128, F:F + 2 * NH])

                # one stage tile PER GATHER: disjoint tiles let the gather
                # ucode instructions run concurrently (whole-tile WAW
                # tracking would serialize slices of a shared tile)
                TL2 = T_LO // 2
                sections = [(0, TL2, 0), (TL2, T_LO, 0), (T_LO, T, HALF)]
                stparts = []
                for si, (t0, tn, roff) in enumerate(sections):
                    nt = tn - t0
                    st = pb.tile([128, nt * GE], DT.bfloat16,
                                 tag=f"stage{si}")
                    srp = st[:].rearrange("p (t g) -> p t g", g=GE)
                    nc.gpsimd.dma_gather(
                        out_ap=srp[:, 0:nt, :],
                        in_ap=h_ext[roff:, :],
                        idxs_ap=gi[:, t0 * 8:tn * 8],
                        num_idxs=nt * 128, num_idxs_reg=nt * 128,
                        elem_size=GE, single_packet=False,
                        queue_num=qs[(3 * b + si) % 4])
                    stparts.append((srp, t0, tn))

                # transposed one-hot selT[m, (t, e)] = (dstL[e, t] == m), bf16
                selT = prhs.tile([128, T * 128], DT.bfloat16, tag="selT")
                selTr = selT[:].rearrange("p (t e) -> p t e", e=128)
                nc.vector.tensor_tensor(
                    out=selTr[:, 0:T, :],
                    in0=dlb[:].rearrange("p (t e) -> p t e", e=128),
                    in1=iota_cw[:].rearrange("p (t e) -> p t e", e=128),
                    op=ALU.is_equal)
                # a_dst[e, h] per edge via one-hot matmul
                par = p2ps.tile([128, T * NH], DT.float32, tag="par")
                parr = par[:].rearrange("p (t e) -> p t e", e=NH)
                for j in range(T):
                    nc.tensor.matmul(
                        out=parr[:, j, :], lhsT=selTr[:, j, :],
                        rhs=a_blk[:, NH:2 * NH], start=True, stop=True)

                # one-hot sel[e, (t, m)] = (dstL[e, t] == m), bf16
                sel = prhs.tile([128, T * 128], DT.bfloat16, tag="sel")
                selr = sel[:].rearrange("p (t m) -> p t m", m=128)
                nc.vector.tensor_tensor(
                    out=selr[:, 0:T, :],
                    in0=dl[:][:, :, None].to_broadcast([128, T, 128]),
                    in1=iota16[:][:, None, :].to_broadcast([128, T, 128]),
                    op=ALU.is_equal)

                rhs = prhs.tile([128, T * RC], DT.bfloat16, tag="rhs")
                rr = rhs[:].rearrange("p (t c) -> p t c", c=RC)

                # ea chain: alpha -> leaky -> exp into rhs[:, :, 128:132]
                scr = p2.tile([128, T * NH], DT.float32, tag="scr")
                scrr = scr[:].rearrange("p (t e) -> p t e", e=NH)
                for srp, t0, tn in stparts:
                    nc.vector.tensor_tensor(
                        out=scrr[:, t0:tn, :],
                        in0=srp[:, 0:tn - t0, F:F + NH],
                        in1=parr[:, t0:tn, :], op=ALU.add)
                scr2 = p2.tile([128, T * NH], DT.float32, tag="scr2")
                scr2r = scr2[:].rearrange("p (t e) -> p t e", e=NH)
                nc.vector.tensor_tensor(
                    out=scr2r[:, 0:T, :], in0=scrr[:, 0:T, :],
                    in1=bass.AP(leak_c[:].tensor, 0,
                                [[1, 128], [0, T], [0, NH]]),
                    op=ALU.mult)
                nc.vector.tensor_tensor(
                    out=scrr[:, 0:T, :], in0=scrr[:, 0:T, :],
                    in1=scr2r[:, 0:T, :], op=ALU.max)
                nc.scalar.activation(out=rr[:, 0:T, F:F + NH],
                                     in_=scrr[:, 0:T, :], func=ACTF.Exp)

                # h copy on scalar engine; Gs = h * ea (per-head broadcast)
                for srp, t0, tn in stparts:
                    nt = tn - t0
                    nc.scalar.copy(out=rr[:, t0:tn, F + NH:RC],
                                   in_=srp[:, 0:nt, 0:F])
                    nc.vector.tensor_tensor(
                        out=rr[:, t0:tn, 0:F].rearrange(
                            "p t (h e) -> p t h e", e=HD),
                        in0=srp[:, 0:nt, 0:F].rearrange(
                            "p t (h e) -> p t h e", e=HD),
                        in1=rr[:, t0:tn, F:F + NH][:, :, :, None]
                        .to_broadcast([128, nt, NH, HD]),
                        op=ALU.mult)

                # accumulate
                acc = p2ps.tile([128, RC], DT.float32, tag="acc")
                for j in range(T):
                    nc.tensor.matmul(
                        out=acc[:], lhsT=selr[:, j, :], rhs=rr[:, j, :],
                        start=(j == 0), stop=(j == T - 1))

                # ---- evac: out = P / s + Q ----
                sden = p2.tile([128, NH], DT.float32, tag="sden")
                nc.vector.tensor_scalar_max(out=sden[:], in0=acc[:, F:F + NH],
                                            scalar1=1e-30)
                rs = p2.tile([128, NH], DT.float32, tag="rs")
                nc.vector.reciprocal(out=rs[:], in_=sden[:])
                ot = p2.tile([128, F], DT.float32, tag="ot")
                otr = ot[:].rearrange("p (h e) -> p h e", e=HD)
                nc.vector.tensor_tensor(
                    out=otr,
                    in0=acc[:, 0:F].rearrange("p (h e) -> p h e", e=HD),
                    in1=rs[:][:, :, None].to_broadcast([128, NH, HD]),
                    op=ALU.mult)
                nc.vector.tensor_tensor(
                    out=otr, in0=otr,
                    in1=acc[:, F + NH:RC].rearrange("p (h e) -> p h e", e=HD),
                    op=ALU.add)
                nc.sync.dma_start(out=out[b * 128:b * 128 + rows, :],
                                  in_=ot[:rows, :])

    return nc


def route_edges(edge_index, N, n_cores, half=32768):
    """Host edge routing. Returns (T_LO, T_HI, per_core index dicts)."""
    src = np.concatenate([np.asarray(edge_index[0]),
                          np.arange(N)]).astype(np.int64)
    dst = np.concatenate([np.asarray(edge_index[1]),
                          np.arange(N)]).astype(np.int64)
    dev_n = N // n_cores
    assert dev_n * n_cores == N
    core = dst // dev_n
    nblk = (dev_n + 127) // 128

    per_core_raw = []
    T_LO = T_HI = 0
    for d in range(n_cores):
        m = core == d
        s_rot = (src[m] - d * dev_n) % N
        d_loc = dst[m] - d * dev_n
        blk = d_loc // 128
        lo = s_rot < half
        cnt_lo = np.bincount(blk[lo], minlength=nblk)
        cnt_hi = np.bincount(blk[~lo], minlength=nblk)
        T_LO = max(T_LO, int(-(-cnt_lo.max() // 128)))
        T_HI = max(T_HI, int(-(-cnt_hi.max() // 128)))
        per_core_raw.append((s_rot, d_loc, blk, lo))
    T_HI = max(T_HI, 1)
    T_LO = max(T_LO, 1)
    T = T_LO + T_HI

    per_core = []
    for d in range(n_cores):
        s_rot, d_loc, blk, lo = per_core_raw[d]
        gidx16 = np.zeros((16, nblk * T * 8), dtype=np.int16)
        dstL = np.full((128, nblk * T), -1, dtype=np.int16)
        for b in range(nblk):
            bcol = b * T * 8
            for sec in (0, 1):
                if sec == 0:
                    bm = (blk == b) & lo
                    voff, t0, sec_col = 0, 0, bcol
                else:
                    bm = (blk == b) & ~lo
                    voff, t0, sec_col = half, T_LO, bcol + T_LO * 8
                vals = s_rot[bm]
                dloc_b = d_loc[bm]
                n = len(vals)
                if n == 0:
                    continue
                order = np.argsort(vals, kind="stable")
                vals = vals[order] - voff
                dloc_b = dloc_b[order]
                jj = np.arange(n)
                gidx16[jj % 16, sec_col + jj // 16] = vals.astype(np.int16)
                dstL[jj % 128, b * T + t0 + jj // 128] = (
                    dloc_b - b * 128).astype(np.int16)

        # dstLb[m, (b, t, e)] = dstL[e, b*T + t], replicated down partitions
        dstLb = np.ascontiguousarray(np.broadcast_to(
            dstL.T.reshape(1, -1), (128, nblk * T * 128)).astype(np.int16))
        per_core.append({
            "gidx": np.tile(gidx16, (8, 1)),
            "dstLb": dstLb,
            "dstL": dstL,
        })
    return T_LO, T_HI, per_core


def host_prep(x, edge_index, W, att_src, att_dst, n_cores, half=32768):
    """Returns (T_LO, T_HI, per-core in_maps list)."""
    N = x.shape[0]
    dev_n = N // n_cores
    bf16 = DT.np(DT.bfloat16)
    xTf = np.ascontiguousarray(np.asarray(x).T.astype(np.float32))
    Wnat = np.ascontiguousarray(np.asarray(W).astype(np.float32))
    Wt = np.ascontiguousarray(Wnat.T)
    A = np.zeros((F, 2 * NH), dtype=np.float32)
    for h in range(NH):
        A[h * HD:(h + 1) * HD, h] = np.asarray(att_src)[0, h]
        A[h * HD:(h + 1) * HD, NH + h] = np.asarray(att_dst)[0, h]
    T_LO, T_HI, per_core = route_edges(edge_index, N, n_cores, half)
    in_maps = []
    for d in range(n_cores):
        xr = np.roll(xTf, -d * dev_n, axis=1)
        in_maps.append(dict(per_core[d],
                            xT16=np.ascontiguousarray(xr.astype(bf16)),
                            Wnat=Wnat, Wt=Wt, Aatt=A))
    return T_LO, T_HI, in_maps


# ---------------------------------------------------------------------------
# Self-contained kernel entry point (full problem size hardcoded).
# ---------------------------------------------------------------------------
N_NODES = 50000
N_CORES = 8
HALF_SPLIT = 32768


def _run(inputs, trace=False):
    import time
    from concourse.bass_utils import run_bass_kernel_spmd

    global LAST_RES
    x = np.asarray(inputs["x"], dtype=np.float32)
    edge_index = np.asarray(inputs["edge_index"])
    W = np.asarray(inputs["W"], dtype=np.float32)
    att_src = np.asarray(inputs["att_src"], dtype=np.float32)
    att_dst = np.asarray(inputs["att_dst"], dtype=np.float32)

    N = x.shape[0]
    assert N == N_NODES, N
    dev_n = N // N_CORES

    t0 = time.time()
    T_LO, T_HI, in_maps = host_prep(x, edge_index, W, att_src, att_dst,
                                    N_CORES, half=HALF_SPLIT)
    t1 = time.time()
    nc = build_gat_nc(N, dev_n, T_LO, T_HI, HALF=HALF_SPLIT)
    nc.compile()
    t2 = time.time()
    res = run_bass_kernel_spmd(nc, in_maps, list(range(N_CORES)), trace=trace)
    LAST_RES = res
    t3 = time.time()
    print(f"kernel: host_prep {t1-t0:.1f}s build+compile {t2-t1:.1f}s "
          f"run {t3-t2:.1f}s T_LO={T_LO} T_HI={T_HI}")
    out = np.concatenate([res.results[d]["out"] for d in range(N_CORES)],
                         axis=0).astype(np.float32)
    return out, res.exec_time_ns


def kernel(**inputs) -> np.ndarray:
    return _run(inputs, trace=False)[0]
